# revision 40
# baseline (speedup 1.0000x reference)
"""AttentionBlock3D TRN2 kernel.

reference: x:[B=2,C=256,D=8,H=32,W=32] -> groupnorm(32 groups) -> qkv 1x1conv
-> full attention over N=D*H*W=8192 positions -> proj 1x1conv -> +x.

Sharding: 8 cores = 2 batches x 4 q-row shards (2048 q positions each).
Each core redundantly computes groupnorm + k/v' for its batch (cheap), then
its q-chunk's attention rows. One SPMD program serves all cores: the host
rotates x along the spatial axis per core so the owned q-chunk is always
columns [0:2048] (attention + groupnorm are invariant to key-position
permutation).

On-device layout (per core):
  x:   [c=128 partitions, t=2 c-tiles, n]  (channel = t*128 + p)
  S^T flash attention: S^T tiles [m=128, nblk] via lhsT=k, rhs=q;
  exp on ScalarE (softmax scale folded into activation scale; no max
  subtraction -- logits are O(6) so exp is safe in fp32);
  PV via lhsT=P^T tile, rhs=v'^T (v' = proj_w @ v, precomputed) augmented
  with a ones column so row-sums accumulate for free in psum col 256.
  Output [n,o] is scaled by 1/rowsum, PE-transposed to [o,n], residual and
  proj bias added, DMA'd out.

All heavy matmuls use float32r (full PE rate at free-dim >= 256,
~1e-3 rel err); stat matmuls use float32.
"""

import sys
from contextlib import ExitStack

import numpy as np

sys.path.insert(0, "/opt/trn_rl_repo")

C = 256
NT = 2           # c-tiles of 128
GROUPS = 32
CPG = C // GROUPS  # channels per group = 8
EPS = 1e-5
SCALE = C ** -0.5


def _patch_ldw_opt():
    import os
    if os.environ.get("KERNEL_LDW_OPT", "0") != "1":
        return
    from concourse import bass_utils
    if getattr(bass_utils, "_ldw_patched", False):
        return
    orig = bass_utils.run_command

    def run_command_ldw(argv, **kwargs):
        argv = [a.replace("--enable-ldw-opt=false", "--enable-ldw-opt=true")
                if isinstance(a, str) else a for a in argv]
        return orig(argv, **kwargs)

    bass_utils.run_command = run_command_ldw
    bass_utils._ldw_patched = True


def build_nc(N=8192, CHUNK=2048, MACRO=512, NBLK=512):
    import concourse.bass as bass
    import concourse.tile as tile
    from concourse import bacc, mybir
    _patch_ldw_opt()

    f32 = mybir.dt.float32
    f32r = mybir.dt.float32r
    Alu = mybir.AluOpType
    Act = mybir.ActivationFunctionType

    NBLK = min(NBLK, CHUNK)
    NMAC = N // MACRO
    MT = N // 128          # number of 128-wide m (key) tiles
    NBLOCKS = CHUNK // NBLK

    nc = bacc.Bacc("TRN2", target_bir_lowering=False, debug=False, num_devices=8)

    x_d = nc.dram_tensor("x", [128, NT, N], f32r, kind="ExternalInput")
    wq_d = nc.dram_tensor("wq", [128, NT, C], f32r, kind="ExternalInput")
    wk_d = nc.dram_tensor("wk", [128, NT, C], f32r, kind="ExternalInput")
    wvp_d = nc.dram_tensor("wvp", [128, NT, C], f32r, kind="ExternalInput")
    qb_d = nc.dram_tensor("qb", [128, NT], f32, kind="ExternalInput")
    kb_d = nc.dram_tensor("kb", [128, NT], f32, kind="ExternalInput")
    pb_d = nc.dram_tensor("pb", [128, NT], f32, kind="ExternalInput")
    gw_d = nc.dram_tensor("gw", [128, NT], f32, kind="ExternalInput")
    gb_d = nc.dram_tensor("gb", [128, NT], f32, kind="ExternalInput")
    ind_d = nc.dram_tensor("ind", [128, NT, GROUPS], f32, kind="ExternalInput")
    indT_d = nc.dram_tensor("indT", [GROUPS, NT, 128], f32, kind="ExternalInput")
    id_d = nc.dram_tensor("ident", [128, 128], f32, kind="ExternalInput")
    out_d = nc.dram_tensor("out", [NT, 128, CHUNK], f32, kind="ExternalOutput")

    with tile.TileContext(nc) as tc, ExitStack() as ctx:
        consts = ctx.enter_context(tc.tile_pool(name="consts", bufs=1))
        big = ctx.enter_context(tc.tile_pool(name="big", bufs=1))
        xst = ctx.enter_context(tc.tile_pool(name="xst", bufs=3))
        work = ctx.enter_context(tc.tile_pool(name="work", bufs=2))
        ptp = ctx.enter_context(tc.tile_pool(name="ptp", bufs=3))
        outp = ctx.enter_context(tc.tile_pool(name="outp", bufs=2))
        small = ctx.enter_context(tc.tile_pool(name="small", bufs=1))
        ps_s = ctx.enter_context(tc.tile_pool(name="ps_s", bufs=4, space="PSUM"))
        ps_pv = ctx.enter_context(tc.tile_pool(name="ps_pv", bufs=1, space="PSUM"))
        ps_qkv = ps_s

        # ---- constants ----
        wq_sb = consts.tile([128, NT, C], f32r)
        wk_sb = consts.tile([128, NT, C], f32r)
        wvp_sb = consts.tile([128, NT, C], f32r)
        for t_sb, t_d in ((wq_sb, wq_d), (wk_sb, wk_d), (wvp_sb, wvp_d)):
            nc.gpsimd.dma_start(t_sb[:], t_d[:, :, :])
        qb_sb = consts.tile([128, NT], f32)
        kb_sb = consts.tile([128, NT], f32)
        pb_sb = consts.tile([128, NT], f32)
        gw_sb = consts.tile([128, NT], f32)
        gb_sb = consts.tile([128, NT], f32)
        for t_sb, t_d in ((qb_sb, qb_d), (kb_sb, kb_d), (pb_sb, pb_d), (gw_sb, gw_d), (gb_sb, gb_d)):
            nc.gpsimd.dma_start(t_sb[:], t_d[:, :])
        ind_sb = consts.tile([128, NT, GROUPS], f32)
        nc.gpsimd.dma_start(ind_sb[:], ind_d[:, :, :])
        indT_sb = consts.tile([GROUPS, NT, 128], f32)
        nc.gpsimd.dma_start(indT_sb[:], indT_d[:, :, :])
        id_sb = consts.tile([128, 128], f32)
        nc.gpsimd.dma_start(id_sb[:], id_d[:, :])
        eps_t = consts.tile([GROUPS, 1], f32)
        nc.vector.memset(eps_t[:], EPS)
        # preload the ln/exp activation tables while the x stream runs
        warm = consts.tile([1, 1], f32)
        nc.scalar.activation(warm[:], eps_t[0:1, :], Act.Ln)
        nc.scalar.activation(warm[:], warm[:], Act.Exp)

        # ---- persistent big buffers ----
        k_sb = big.tile([128, NT, N], f32r)
        q_sb = big.tile([128, NT, CHUNK], f32r)
        vpT = big.tile([128, MT, C + 2], f32r)
        nc.vector.memset(vpT[:, :, C:C + 1].bitcast(f32), 1.0)
        nc.vector.memset(vpT[:, :, C + 1:C + 2].bitcast(f32), 0.0)

        # ================= Phase A: groupnorm stats =================
        SCH = 512
        NSC = N // SCH
        st_all = big.tile([128, NT, NSC, 6], f32)
        for im in reversed(range(NSC)):
            xa = xst.tile([128, NT, SCH], f32r, tag="xa", bufs=4)
            nc.sync.dma_start(xa[:], x_d[:, :, bass.ts(im, SCH)])
            for t in range(NT):
                nc.vector.bn_stats(out=st_all[:, t, im, :], in_=xa[:, t, :])
        mv = big.tile([128, NT, 2], f32)
        # per-channel (mean, E[x^2]); group-reduce via indicator matmul
        for t in range(NT):
            nc.vector.bn_aggr(out=mv[:, t, :], in_=st_all[:, t, :, :])
        sq = small.tile([128, NT, 1], f32, tag="sq")
        nc.vector.tensor_mul(sq[:], mv[:, :, 0:1], mv[:, :, 0:1])
        nc.vector.tensor_add(mv[:, :, 1:2], mv[:, :, 1:2], sq[:])
        gsps = []
        for t in range(NT):
            gsp_t = ps_qkv.tile([GROUPS, 2], f32, tag="sp", name=f"gsp{t}")
            nc.tensor.matmul(gsp_t[:], ind_sb[:, t, :], mv[:, t, :],
                             start=True, stop=True)
            gsps.append(gsp_t)
        gsum = small.tile([GROUPS, 2], f32, tag="gsum")
        nc.vector.tensor_copy(gsum[:], gsps[0][:])
        nc.vector.tensor_add(gsum[:], gsum[:], gsps[1][:])
        gm = small.tile([GROUPS, 1], f32, tag="gm")
        ge2 = small.tile([GROUPS, 1], f32, tag="ge2")
        nc.vector.tensor_scalar_mul(gm[:], gsum[:, 0:1], 1.0 / CPG)
        nc.vector.tensor_scalar_mul(ge2[:], gsum[:, 1:2], 1.0 / CPG)
        gm2 = small.tile([GROUPS, 1], f32, tag="gm2")
        nc.vector.tensor_mul(gm2[:], gm[:], gm[:])
        gvar = small.tile([GROUPS, 1], f32, tag="gvar")
        nc.vector.tensor_sub(gvar[:], ge2[:], gm2[:])
        # rstd = exp(-0.5 * ln(var + eps))  (sqrt activation is too imprecise)
        lnv = small.tile([GROUPS, 1], f32, tag="lnv")
        nc.scalar.activation(lnv[:], gvar[:], Act.Ln, bias=eps_t[:], scale=1.0)
        grs = small.tile([GROUPS, 1], f32, tag="grs")
        nc.scalar.activation(grs[:], lnv[:], Act.Exp, scale=-0.5)
        gsb = small.tile([GROUPS, 2], f32, tag="gsb")
        nc.vector.tensor_copy(gsb[:, 0:1], gm[:])
        nc.vector.tensor_copy(gsb[:, 1:2], grs[:])
        # broadcast to channels; fold into per-channel affine h = A*x + B
        ab = big.tile([128, NT, 2], f32)
        for t in range(NT):
            mrp = ps_qkv.tile([128, 2], f32, tag="sp")
            nc.tensor.matmul(mrp[:], indT_sb[:, t, :], gsb[:], start=True, stop=True)
            tmp = small.tile([128, 1], f32, tag="tmpab")
            nc.vector.tensor_mul(ab[:, t, 0:1], mrp[:, 1:2], gw_sb[:, t:t + 1])
            nc.vector.tensor_mul(tmp[:], mrp[:, 0:1], ab[:, t, 0:1])
            nc.vector.tensor_sub(ab[:, t, 1:2], gb_sb[:, t:t + 1], tmp[:])

        # ================= Phase B: h -> k, q, v'^T =================
        def load_and_normalize(im):
            xt = xst.tile([128, NT, MACRO], f32r, tag="xa", bufs=4,
                          name=f"xt_{im}")
            nc.sync.dma_start(xt[:], x_d[:, :, bass.ts(im, MACRO)])
            ht = work.tile([128, NT, MACRO], f32r, tag="ht", bufs=3,
                           name=f"ht_{im}")
            for t in range(NT):
                nc.vector.tensor_scalar(
                    out=ht[:, t, :], in0=xt[:, t, :],
                    scalar1=ab[:, t, 0:1], scalar2=ab[:, t, 1:2],
                    op0=Alu.mult, op1=Alu.add)
            return ht

        ht_next = load_and_normalize(0)
        for im in range(NMAC):
            mb = im * MACRO
            ht = ht_next
            if im + 1 < NMAC:
                ht_next = load_and_normalize(im + 1)
            # k = Wk @ h + kb   (k_sb[:, oc, :] in [o, m] layout)
            for oc in range(NT):
                kp = ps_qkv.tile([128, MACRO], f32, tag="sp")
                for t in range(NT):
                    nc.tensor.matmul(kp[:], wk_sb[:, t, bass.ts(oc, 128)],
                                     ht[:, t, :], start=(t == 0), stop=(t == NT - 1))
                nc.vector.tensor_scalar_add(
                    k_sb[:, oc, bass.ts(im, MACRO)], kp[:], kb_sb[:, oc:oc + 1])
            # q only for owned chunk (columns [0, CHUNK))
            qlo = max(mb, 0)
            qhi = min(mb + MACRO, CHUNK)
            if qlo < qhi:
                qn = qhi - qlo
                for oc in range(NT):
                    qp = ps_qkv.tile([128, MACRO], f32, tag="sp")
                    for t in range(NT):
                        nc.tensor.matmul(qp[:, :qn],
                                         wq_sb[:, t, bass.ts(oc, 128)],
                                         ht[:, t, qlo - mb:qhi - mb],
                                         start=(t == 0), stop=(t == NT - 1))
                    nc.vector.tensor_scalar_add(
                        q_sb[:, oc, qlo:qhi], qp[:, :qn], qb_sb[:, oc:oc + 1])
            # v'^T tiles: v'T[m, o] = sum_c h[c, m] * wvp[c, o]
            for mm in range(MACRO // 128):
                j = im * (MACRO // 128) + mm
                vpp = ps_qkv.tile([128, C], f32, tag="sp")
                for t in range(NT):
                    nc.tensor.matmul(vpp[:], ht[:, t, bass.ts(mm, 128)],
                                     wvp_sb[:, t, :], start=(t == 0), stop=(t == NT - 1))
                nc.scalar.copy(vpT[:, j, 0:C], vpp[:])

        # ================= Phase C: attention per n-block =================
        NH = NBLK // 128
        for blk in range(NBLOCKS):
            nb = blk * NBLK
            pvs = [ps_pv.tile([128, C + 2], f32, tag=f"pv{nh}", name=f"pv{nh}_{blk}") for nh in range(NH)]
            for j in range(MT):
                sp = ps_s.tile([128, NBLK], f32, tag="sp")
                for t in range(NT):
                    nc.tensor.matmul(sp[:],
                                     k_sb[:, t, bass.ts(j, 128)],
                                     q_sb[:, t, nb:nb + NBLK],
                                     start=(t == 0), stop=(t == NT - 1))
                pt = ptp.tile([128, NBLK], f32r, tag="pt", bufs=3)
                for eh in range(2):
                    nc.scalar.activation(pt[:, bass.ts(eh, NBLK // 2)],
                                         sp[:, bass.ts(eh, NBLK // 2)],
                                         Act.Exp, scale=SCALE)
                    for nh in range(eh * NH // 2, (eh + 1) * NH // 2):
                        nc.tensor.matmul(pvs[nh][:],
                                         pt[:, bass.ts(nh, 128)],
                                         vpT[:, j, :],
                                         start=(j == 0), stop=(j == MT - 1))
            # finalize: scale rows by 1/rowsum, transpose to [o, n], +bias +x
            xres = outp.tile([128, NT, NBLK], f32r, tag="xres", bufs=1)
            nc.sync.dma_start(xres[:], x_d[:, :, nb:nb + NBLK])
            for oc in range(NT):
                nc.vector.tensor_scalar_add(xres[:, oc, :].bitcast(f32),
                                            xres[:, oc, :].bitcast(f32),
                                            pb_sb[:, oc:oc + 1])
            outT = outp.tile([128, NH, C], f32, tag="outT", bufs=1)
            for nh in range(NH):
                rec = small.tile([128, 1], f32, tag="rec", bufs=2)
                nc.vector.reciprocal(rec[:], pvs[nh][:, C:C + 1])
                nc.scalar.activation(outT[:, nh, :], pvs[nh][:, 0:C],
                                     Act.Copy, scale=rec[:])
            ob = outp.tile([128, NT, NBLK], f32, tag="ob")
            for oc in range(NT):
                for nh in range(NH):
                    tp = ps_s.tile([128, 128], f32, tag="sp")
                    nc.tensor.transpose(tp[:], outT[:, nh, bass.ts(oc, 128)], id_sb[:])
                    nc.vector.tensor_add(ob[:, oc, bass.ts(nh, 128)], tp[:],
                                         xres[:, oc, bass.ts(nh, 128)].bitcast(f32))
                nc.sync.dma_start(out_d[oc, :, nb:nb + NBLK], ob[:, oc, :])

    nc.compile()
    return nc


def prep_inputs(xf_b_rot, qkv_w, qkv_b, proj_w, proj_b, norm_w, norm_b, N):
    """Per-core input map. xf_b_rot: [C, N] rotated so owned chunk is cols 0:."""
    def ct(a):  # [C, ...] -> [128, NT, ...] with channel = t*128 + p
        return np.ascontiguousarray(a.reshape(NT, 128, *a.shape[1:]).transpose(
            1, 0, *range(2, a.ndim + 1)))

    wq = qkv_w[0:C].T          # [c, o]
    wk = qkv_w[C:2 * C].T
    wvp = (proj_w.astype(np.float64) @ qkv_w[2 * C:3 * C].astype(np.float64)
           ).astype(np.float32).T
    pbm = (proj_b.astype(np.float64)
           + proj_w.astype(np.float64) @ qkv_b[2 * C:3 * C].astype(np.float64)
           ).astype(np.float32)
    ind = np.zeros((C, GROUPS), np.float32)
    ind[np.arange(C), np.arange(C) // CPG] = 1.0
    indT = np.ascontiguousarray(
        ind.reshape(NT, 128, GROUPS).transpose(2, 0, 1))  # [g, t, p]
    return {
        "x": ct(xf_b_rot.astype(np.float32)),
        "wq": ct(wq.astype(np.float32)), "wk": ct(wk.astype(np.float32)),
        "wvp": ct(wvp),
        "qb": ct(qkv_b[0:C].astype(np.float32)),
        "kb": ct(qkv_b[C:2 * C].astype(np.float32)),
        "pb": ct(pbm),
        "gw": ct(norm_w.astype(np.float32)), "gb": ct(norm_b.astype(np.float32)),
        "ind": ct(ind), "indT": indT,
        "ident": np.eye(128, dtype=np.float32),
    }


_NC_CACHE = {}


def _get_nc(N, CHUNK):
    key = (N, CHUNK)
    if key not in _NC_CACHE:
        _NC_CACHE[key] = build_nc(N=N, CHUNK=CHUNK)
    return _NC_CACHE[key]


def run_cores(inputs, N=8192, CHUNK=2048, trace=False):
    """Shard, run on 8 cores, return (out [B, C, N], BassKernelResults)."""
    from concourse.bass_utils import run_bass_kernel_spmd

    x = np.asarray(inputs["x"], np.float32)
    B = x.shape[0]
    xf = x.reshape(B, C, N)
    qshards = 8 // B
    nc = _get_nc(N, CHUNK)
    in_maps = []
    for core in range(8):
        b, ci = divmod(core, qshards)
        s = ci * CHUNK
        xrot = np.concatenate([xf[b][:, s:], xf[b][:, :s]], axis=1)
        in_maps.append(prep_inputs(
            xrot, np.asarray(inputs["qkv_w"], np.float32),
            np.asarray(inputs["qkv_b"], np.float32),
            np.asarray(inputs["proj_w"], np.float32),
            np.asarray(inputs["proj_b"], np.float32),
            np.asarray(inputs["norm_w"], np.float32),
            np.asarray(inputs["norm_b"], np.float32), N))
    res = run_bass_kernel_spmd(nc, in_maps, core_ids=list(range(8)), trace=trace)
    out = np.empty((B, C, N), np.float32)
    for core in range(8):
        b, ci = divmod(core, qshards)
        out[b][:, ci * CHUNK:(ci + 1) * CHUNK] = res.results[core]["out"].reshape(
            C, CHUNK)
    return out, res


def kernel(x, norm_w, norm_b, qkv_w, qkv_b, proj_w, proj_b):
    B, _, D, H, W = x.shape
    out, _ = run_cores(
        dict(x=x, norm_w=norm_w, norm_b=norm_b, qkv_w=qkv_w, qkv_b=qkv_b,
             proj_w=proj_w, proj_b=proj_b),
        N=D * H * W, CHUNK=D * H * W * B // 8)
    return out.reshape(B, C, D, H, W).astype(np.float32)


# revision 43
# speedup vs baseline: 1.0084x; 1.0084x over previous
"""AttentionBlock3D TRN2 kernel.

reference: x:[B=2,C=256,D=8,H=32,W=32] -> groupnorm(32 groups) -> qkv 1x1conv
-> full attention over N=D*H*W=8192 positions -> proj 1x1conv -> +x.

Sharding: 8 cores = 2 batches x 4 q-row shards (2048 q positions each).
Each core redundantly computes groupnorm + k/v' for its batch (cheap), then
its q-chunk's attention rows. One SPMD program serves all cores: the host
rotates x along the spatial axis per core so the owned q-chunk is always
columns [0:2048] (attention + groupnorm are invariant to key-position
permutation).

On-device layout (per core):
  x:   [c=128 partitions, t=2 c-tiles, n]  (channel = t*128 + p)
  S^T flash attention: S^T tiles [m=128, nblk] via lhsT=k, rhs=q;
  exp on ScalarE (softmax scale folded into activation scale; no max
  subtraction -- logits are O(6) so exp is safe in fp32);
  PV via lhsT=P^T tile, rhs=v'^T (v' = proj_w @ v, precomputed) augmented
  with a ones column so row-sums accumulate for free in psum col 256.
  Output [n,o] is scaled by 1/rowsum, PE-transposed to [o,n], residual and
  proj bias added, DMA'd out.

All heavy matmuls use float32r (full PE rate at free-dim >= 256,
~1e-3 rel err); stat matmuls use float32.
"""

import sys
from contextlib import ExitStack

import numpy as np

sys.path.insert(0, "/opt/trn_rl_repo")

C = 256
NT = 2           # c-tiles of 128
GROUPS = 32
CPG = C // GROUPS  # channels per group = 8
EPS = 1e-5
SCALE = C ** -0.5


def _patch_ldw_opt():
    import os
    if os.environ.get("KERNEL_LDW_OPT", "0") != "1":
        return
    from concourse import bass_utils
    if getattr(bass_utils, "_ldw_patched", False):
        return
    orig = bass_utils.run_command

    def run_command_ldw(argv, **kwargs):
        argv = [a.replace("--enable-ldw-opt=false", "--enable-ldw-opt=true")
                if isinstance(a, str) else a for a in argv]
        return orig(argv, **kwargs)

    bass_utils.run_command = run_command_ldw
    bass_utils._ldw_patched = True


def build_nc(N=8192, CHUNK=2048, MACRO=512, NBLK=512):
    import concourse.bass as bass
    import concourse.tile as tile
    from concourse import bacc, mybir
    _patch_ldw_opt()

    f32 = mybir.dt.float32
    f32r = mybir.dt.float32r
    Alu = mybir.AluOpType
    Act = mybir.ActivationFunctionType

    NBLK = min(NBLK, CHUNK)
    NMAC = N // MACRO
    MT = N // 128          # number of 128-wide m (key) tiles
    NBLOCKS = CHUNK // NBLK

    nc = bacc.Bacc("TRN2", target_bir_lowering=False, debug=False, num_devices=8)

    x_d = nc.dram_tensor("x", [128, NT, N], f32r, kind="ExternalInput")
    wq_d = nc.dram_tensor("wq", [128, NT, C], f32r, kind="ExternalInput")
    wk_d = nc.dram_tensor("wk", [128, NT, C], f32r, kind="ExternalInput")
    wvp_d = nc.dram_tensor("wvp", [128, NT, C], f32r, kind="ExternalInput")
    qb_d = nc.dram_tensor("qb", [128, NT], f32, kind="ExternalInput")
    kb_d = nc.dram_tensor("kb", [128, NT], f32, kind="ExternalInput")
    pb_d = nc.dram_tensor("pb", [128, NT], f32, kind="ExternalInput")
    gw_d = nc.dram_tensor("gw", [128, NT], f32, kind="ExternalInput")
    gb_d = nc.dram_tensor("gb", [128, NT], f32, kind="ExternalInput")
    ind_d = nc.dram_tensor("ind", [128, NT, GROUPS], f32, kind="ExternalInput")
    indT_d = nc.dram_tensor("indT", [GROUPS, NT, 128], f32, kind="ExternalInput")
    id_d = nc.dram_tensor("ident", [128, 128], f32, kind="ExternalInput")
    out_d = nc.dram_tensor("out", [NT, 128, CHUNK], f32, kind="ExternalOutput")

    with tile.TileContext(nc) as tc, ExitStack() as ctx:
        consts = ctx.enter_context(tc.tile_pool(name="consts", bufs=1))
        big = ctx.enter_context(tc.tile_pool(name="big", bufs=1))
        xst = ctx.enter_context(tc.tile_pool(name="xst", bufs=3))
        work = ctx.enter_context(tc.tile_pool(name="work", bufs=2))
        ptp = ctx.enter_context(tc.tile_pool(name="ptp", bufs=3))
        outp = ctx.enter_context(tc.tile_pool(name="outp", bufs=2))
        small = ctx.enter_context(tc.tile_pool(name="small", bufs=1))
        ps_s = ctx.enter_context(tc.tile_pool(name="ps_s", bufs=4, space="PSUM"))
        ps_pv = ctx.enter_context(tc.tile_pool(name="ps_pv", bufs=1, space="PSUM"))
        ps_qkv = ps_s

        # ---- constants ----
        wq_sb = consts.tile([128, NT, C], f32r)
        wk_sb = consts.tile([128, NT, C], f32r)
        wvp_sb = consts.tile([128, NT, C], f32r)
        for t_sb, t_d in ((wq_sb, wq_d), (wk_sb, wk_d), (wvp_sb, wvp_d)):
            nc.gpsimd.dma_start(t_sb[:], t_d[:, :, :])
        qb_sb = consts.tile([128, NT], f32)
        kb_sb = consts.tile([128, NT], f32)
        pb_sb = consts.tile([128, NT], f32)
        gw_sb = consts.tile([128, NT], f32)
        gb_sb = consts.tile([128, NT], f32)
        for t_sb, t_d in ((qb_sb, qb_d), (kb_sb, kb_d), (pb_sb, pb_d), (gw_sb, gw_d), (gb_sb, gb_d)):
            nc.gpsimd.dma_start(t_sb[:], t_d[:, :])
        ind_sb = consts.tile([128, NT, GROUPS], f32)
        nc.gpsimd.dma_start(ind_sb[:], ind_d[:, :, :])
        indT_sb = consts.tile([GROUPS, NT, 128], f32)
        nc.gpsimd.dma_start(indT_sb[:], indT_d[:, :, :])
        id_sb = consts.tile([128, 128], f32)
        nc.gpsimd.dma_start(id_sb[:], id_d[:, :])
        eps_t = consts.tile([GROUPS, 1], f32)
        nc.vector.memset(eps_t[:], EPS)
        # preload the ln/exp activation tables while the x stream runs
        warm = consts.tile([1, 1], f32)
        nc.scalar.activation(warm[:], eps_t[0:1, :], Act.Ln)
        nc.scalar.activation(warm[:], warm[:], Act.Exp)

        # ---- persistent big buffers ----
        k_sb = big.tile([128, NT, N], f32r)
        q_sb = big.tile([128, NT, CHUNK], f32r)
        vpT = big.tile([128, MT, C + 2], f32r)
        nc.vector.memset(vpT[:, :, C:C + 1].bitcast(f32), 1.0)
        nc.vector.memset(vpT[:, :, C + 1:C + 2].bitcast(f32), 0.0)

        # ================= Phase A: groupnorm stats =================
        SCH = 512
        NSC = N // SCH
        st_all = big.tile([128, NT, NSC, 6], f32)
        for im in reversed(range(NSC)):
            xa = xst.tile([128, NT, SCH], f32r, tag="xa", bufs=5)
            nc.sync.dma_start(xa[:], x_d[:, :, bass.ts(im, SCH)])
            for t in range(NT):
                nc.vector.bn_stats(out=st_all[:, t, im, :], in_=xa[:, t, :])
        mv = big.tile([128, NT, 2], f32)
        # per-channel (mean, E[x^2]); group-reduce via indicator matmul
        for t in range(NT):
            nc.vector.bn_aggr(out=mv[:, t, :], in_=st_all[:, t, :, :])
        sq = small.tile([128, NT, 1], f32, tag="sq")
        nc.vector.tensor_mul(sq[:], mv[:, :, 0:1], mv[:, :, 0:1])
        nc.vector.tensor_add(mv[:, :, 1:2], mv[:, :, 1:2], sq[:])
        gsps = []
        for t in range(NT):
            gsp_t = ps_qkv.tile([GROUPS, 2], f32, tag="sp", name=f"gsp{t}")
            nc.tensor.matmul(gsp_t[:], ind_sb[:, t, :], mv[:, t, :],
                             start=True, stop=True)
            gsps.append(gsp_t)
        gsum = small.tile([GROUPS, 2], f32, tag="gsum")
        nc.vector.tensor_copy(gsum[:], gsps[0][:])
        nc.vector.tensor_add(gsum[:], gsum[:], gsps[1][:])
        gm = small.tile([GROUPS, 1], f32, tag="gm")
        ge2 = small.tile([GROUPS, 1], f32, tag="ge2")
        nc.vector.tensor_scalar_mul(gm[:], gsum[:, 0:1], 1.0 / CPG)
        nc.vector.tensor_scalar_mul(ge2[:], gsum[:, 1:2], 1.0 / CPG)
        gm2 = small.tile([GROUPS, 1], f32, tag="gm2")
        nc.vector.tensor_mul(gm2[:], gm[:], gm[:])
        gvar = small.tile([GROUPS, 1], f32, tag="gvar")
        nc.vector.tensor_sub(gvar[:], ge2[:], gm2[:])
        # rstd = exp(-0.5 * ln(var + eps))  (sqrt activation is too imprecise)
        lnv = small.tile([GROUPS, 1], f32, tag="lnv")
        nc.scalar.activation(lnv[:], gvar[:], Act.Ln, bias=eps_t[:], scale=1.0)
        grs = small.tile([GROUPS, 1], f32, tag="grs")
        nc.scalar.activation(grs[:], lnv[:], Act.Exp, scale=-0.5)
        gsb = small.tile([GROUPS, 2], f32, tag="gsb")
        nc.vector.tensor_copy(gsb[:, 0:1], gm[:])
        nc.vector.tensor_copy(gsb[:, 1:2], grs[:])
        # broadcast to channels; fold into per-channel affine h = A*x + B
        ab = big.tile([128, NT, 2], f32)
        for t in range(NT):
            mrp = ps_qkv.tile([128, 2], f32, tag="sp")
            nc.tensor.matmul(mrp[:], indT_sb[:, t, :], gsb[:], start=True, stop=True)
            tmp = small.tile([128, 1], f32, tag="tmpab")
            nc.vector.tensor_mul(ab[:, t, 0:1], mrp[:, 1:2], gw_sb[:, t:t + 1])
            nc.vector.tensor_mul(tmp[:], mrp[:, 0:1], ab[:, t, 0:1])
            nc.vector.tensor_sub(ab[:, t, 1:2], gb_sb[:, t:t + 1], tmp[:])

        # ================= Phase B: h -> k, q, v'^T =================
        def load_and_normalize(im):
            xt = xst.tile([128, NT, MACRO], f32r, tag="xa", bufs=5,
                          name=f"xt_{im}")
            nc.sync.dma_start(xt[:], x_d[:, :, bass.ts(im, MACRO)])
            ht = work.tile([128, NT, MACRO], f32r, tag="ht", bufs=3,
                           name=f"ht_{im}")
            for t in range(NT):
                nc.vector.tensor_scalar(
                    out=ht[:, t, :], in0=xt[:, t, :],
                    scalar1=ab[:, t, 0:1], scalar2=ab[:, t, 1:2],
                    op0=Alu.mult, op1=Alu.add)
            return ht

        ht_next = load_and_normalize(0)
        for im in range(NMAC):
            mb = im * MACRO
            ht = ht_next
            if im + 1 < NMAC:
                ht_next = load_and_normalize(im + 1)
            # k = Wk @ h + kb   (k_sb[:, oc, :] in [o, m] layout)
            for oc in range(NT):
                kp = ps_qkv.tile([128, MACRO], f32, tag="sp")
                for t in range(NT):
                    nc.tensor.matmul(kp[:], wk_sb[:, t, bass.ts(oc, 128)],
                                     ht[:, t, :], start=(t == 0), stop=(t == NT - 1))
                nc.vector.tensor_scalar_add(
                    k_sb[:, oc, bass.ts(im, MACRO)], kp[:], kb_sb[:, oc:oc + 1])
            # q only for owned chunk (columns [0, CHUNK))
            qlo = max(mb, 0)
            qhi = min(mb + MACRO, CHUNK)
            if qlo < qhi:
                qn = qhi - qlo
                for oc in range(NT):
                    qp = ps_qkv.tile([128, MACRO], f32, tag="sp")
                    for t in range(NT):
                        nc.tensor.matmul(qp[:, :qn],
                                         wq_sb[:, t, bass.ts(oc, 128)],
                                         ht[:, t, qlo - mb:qhi - mb],
                                         start=(t == 0), stop=(t == NT - 1))
                    nc.vector.tensor_scalar_add(
                        q_sb[:, oc, qlo:qhi], qp[:, :qn], qb_sb[:, oc:oc + 1])
            # v'^T tiles: v'T[m, o] = sum_c h[c, m] * wvp[c, o]
            for mm in range(MACRO // 128):
                j = im * (MACRO // 128) + mm
                vpp = ps_qkv.tile([128, C], f32, tag="sp")
                for t in range(NT):
                    nc.tensor.matmul(vpp[:], ht[:, t, bass.ts(mm, 128)],
                                     wvp_sb[:, t, :], start=(t == 0), stop=(t == NT - 1))
                nc.scalar.copy(vpT[:, j, 0:C], vpp[:])

        # ================= Phase C: attention per n-block =================
        NH = NBLK // 128
        for blk in range(NBLOCKS):
            nb = blk * NBLK
            pvs = [ps_pv.tile([128, C + 2], f32, tag=f"pv{nh}", name=f"pv{nh}_{blk}") for nh in range(NH)]
            for j in range(MT):
                sp = ps_s.tile([128, NBLK], f32, tag="sp")
                for t in range(NT):
                    nc.tensor.matmul(sp[:],
                                     k_sb[:, t, bass.ts(j, 128)],
                                     q_sb[:, t, nb:nb + NBLK],
                                     start=(t == 0), stop=(t == NT - 1))
                pt = ptp.tile([128, NBLK], f32r, tag="pt", bufs=3)
                for eh in range(2):
                    nc.scalar.activation(pt[:, bass.ts(eh, NBLK // 2)],
                                         sp[:, bass.ts(eh, NBLK // 2)],
                                         Act.Exp, scale=SCALE)
                    for nh in range(eh * NH // 2, (eh + 1) * NH // 2):
                        nc.tensor.matmul(pvs[nh][:],
                                         pt[:, bass.ts(nh, 128)],
                                         vpT[:, j, :],
                                         start=(j == 0), stop=(j == MT - 1))
            # finalize: scale rows by 1/rowsum, transpose to [o, n], +bias +x
            xres = outp.tile([128, NT, NBLK], f32r, tag="xres", bufs=1)
            nc.sync.dma_start(xres[:], x_d[:, :, nb:nb + NBLK])
            for oc in range(NT):
                nc.vector.tensor_scalar_add(xres[:, oc, :].bitcast(f32),
                                            xres[:, oc, :].bitcast(f32),
                                            pb_sb[:, oc:oc + 1])
            outT = outp.tile([128, NH, C], f32, tag="outT", bufs=1)
            for nh in range(NH):
                rec = small.tile([128, 1], f32, tag="rec", bufs=2)
                nc.vector.reciprocal(rec[:], pvs[nh][:, C:C + 1])
                nc.scalar.activation(outT[:, nh, :], pvs[nh][:, 0:C],
                                     Act.Copy, scale=rec[:])
            ob = outp.tile([128, NT, NBLK], f32, tag="ob")
            for oc in range(NT):
                for nh in range(NH):
                    tp = ps_s.tile([128, 128], f32, tag="sp")
                    nc.tensor.transpose(tp[:], outT[:, nh, bass.ts(oc, 128)], id_sb[:])
                    nc.vector.tensor_add(ob[:, oc, bass.ts(nh, 128)], tp[:],
                                         xres[:, oc, bass.ts(nh, 128)].bitcast(f32))
                nc.sync.dma_start(out_d[oc, :, nb:nb + NBLK], ob[:, oc, :])

    nc.compile()
    return nc


def prep_inputs(xf_b_rot, qkv_w, qkv_b, proj_w, proj_b, norm_w, norm_b, N):
    """Per-core input map. xf_b_rot: [C, N] rotated so owned chunk is cols 0:."""
    def ct(a):  # [C, ...] -> [128, NT, ...] with channel = t*128 + p
        return np.ascontiguousarray(a.reshape(NT, 128, *a.shape[1:]).transpose(
            1, 0, *range(2, a.ndim + 1)))

    wq = qkv_w[0:C].T          # [c, o]
    wk = qkv_w[C:2 * C].T
    wvp = (proj_w.astype(np.float64) @ qkv_w[2 * C:3 * C].astype(np.float64)
           ).astype(np.float32).T
    pbm = (proj_b.astype(np.float64)
           + proj_w.astype(np.float64) @ qkv_b[2 * C:3 * C].astype(np.float64)
           ).astype(np.float32)
    ind = np.zeros((C, GROUPS), np.float32)
    ind[np.arange(C), np.arange(C) // CPG] = 1.0
    indT = np.ascontiguousarray(
        ind.reshape(NT, 128, GROUPS).transpose(2, 0, 1))  # [g, t, p]
    return {
        "x": ct(xf_b_rot.astype(np.float32)),
        "wq": ct(wq.astype(np.float32)), "wk": ct(wk.astype(np.float32)),
        "wvp": ct(wvp),
        "qb": ct(qkv_b[0:C].astype(np.float32)),
        "kb": ct(qkv_b[C:2 * C].astype(np.float32)),
        "pb": ct(pbm),
        "gw": ct(norm_w.astype(np.float32)), "gb": ct(norm_b.astype(np.float32)),
        "ind": ct(ind), "indT": indT,
        "ident": np.eye(128, dtype=np.float32),
    }


_NC_CACHE = {}


def _get_nc(N, CHUNK):
    key = (N, CHUNK)
    if key not in _NC_CACHE:
        _NC_CACHE[key] = build_nc(N=N, CHUNK=CHUNK)
    return _NC_CACHE[key]


def run_cores(inputs, N=8192, CHUNK=2048, trace=False):
    """Shard, run on 8 cores, return (out [B, C, N], BassKernelResults)."""
    from concourse.bass_utils import run_bass_kernel_spmd

    x = np.asarray(inputs["x"], np.float32)
    B = x.shape[0]
    xf = x.reshape(B, C, N)
    qshards = 8 // B
    nc = _get_nc(N, CHUNK)
    in_maps = []
    for core in range(8):
        b, ci = divmod(core, qshards)
        s = ci * CHUNK
        xrot = np.concatenate([xf[b][:, s:], xf[b][:, :s]], axis=1)
        in_maps.append(prep_inputs(
            xrot, np.asarray(inputs["qkv_w"], np.float32),
            np.asarray(inputs["qkv_b"], np.float32),
            np.asarray(inputs["proj_w"], np.float32),
            np.asarray(inputs["proj_b"], np.float32),
            np.asarray(inputs["norm_w"], np.float32),
            np.asarray(inputs["norm_b"], np.float32), N))
    res = run_bass_kernel_spmd(nc, in_maps, core_ids=list(range(8)), trace=trace)
    out = np.empty((B, C, N), np.float32)
    for core in range(8):
        b, ci = divmod(core, qshards)
        out[b][:, ci * CHUNK:(ci + 1) * CHUNK] = res.results[core]["out"].reshape(
            C, CHUNK)
    return out, res


def kernel(x, norm_w, norm_b, qkv_w, qkv_b, proj_w, proj_b):
    B, _, D, H, W = x.shape
    out, _ = run_cores(
        dict(x=x, norm_w=norm_w, norm_b=norm_b, qkv_w=qkv_w, qkv_b=qkv_b,
             proj_w=proj_w, proj_b=proj_b),
        N=D * H * W, CHUNK=D * H * W * B // 8)
    return out.reshape(B, C, D, H, W).astype(np.float32)


# revision 44
# speedup vs baseline: 1.1922x; 1.1823x over previous
"""AttentionBlock3D TRN2 kernel.

reference: x:[B=2,C=256,D=8,H=32,W=32] -> groupnorm(32 groups) -> qkv 1x1conv
-> full attention over N=D*H*W=8192 positions -> proj 1x1conv -> +x.

Sharding: 8 cores = 2 batches x 4 q-row shards (2048 q positions each).
Each core redundantly computes groupnorm + k/v' for its batch (cheap), then
its q-chunk's attention rows. One SPMD program serves all cores: the host
rotates x along the spatial axis per core so the owned q-chunk is always
columns [0:2048] (attention + groupnorm are invariant to key-position
permutation).

On-device layout (per core):
  x:   [c=128 partitions, t=2 c-tiles, n]  (channel = t*128 + p)
  S^T flash attention: S^T tiles [m=128, nblk] via lhsT=k, rhs=q;
  exp on ScalarE (softmax scale folded into activation scale; no max
  subtraction -- logits are O(6) so exp is safe in fp32);
  PV via lhsT=P^T tile, rhs=v'^T (v' = proj_w @ v, precomputed) augmented
  with a ones column so row-sums accumulate for free in psum col 256.
  Output [n,o] is scaled by 1/rowsum, PE-transposed to [o,n], residual and
  proj bias added, DMA'd out.

All heavy matmuls use float32r (full PE rate at free-dim >= 256,
~1e-3 rel err); stat matmuls use float32.
"""

import sys
from contextlib import ExitStack

import numpy as np

sys.path.insert(0, "/opt/trn_rl_repo")

C = 256
NT = 2           # c-tiles of 128
GROUPS = 32
CPG = C // GROUPS  # channels per group = 8
EPS = 1e-5
SCALE = C ** -0.5


def _patch_ldw_opt():
    import os
    if os.environ.get("KERNEL_LDW_OPT", "0") != "1":
        return
    from concourse import bass_utils
    if getattr(bass_utils, "_ldw_patched", False):
        return
    orig = bass_utils.run_command

    def run_command_ldw(argv, **kwargs):
        argv = [a.replace("--enable-ldw-opt=false", "--enable-ldw-opt=true")
                if isinstance(a, str) else a for a in argv]
        return orig(argv, **kwargs)

    bass_utils.run_command = run_command_ldw
    bass_utils._ldw_patched = True


def build_nc(N=8192, CHUNK=2048, MACRO=512, NBLK=512):
    import concourse.bass as bass
    import concourse.tile as tile
    from concourse import bacc, mybir
    _patch_ldw_opt()

    f32 = mybir.dt.float32
    f32r = mybir.dt.float32r
    Alu = mybir.AluOpType
    Act = mybir.ActivationFunctionType

    NBLK = min(NBLK, CHUNK)
    NMAC = N // MACRO
    MT = N // 128          # number of 128-wide m (key) tiles
    NBLOCKS = CHUNK // NBLK

    nc = bacc.Bacc("TRN2", target_bir_lowering=False, debug=False, num_devices=8)

    x_d = nc.dram_tensor("x", [128, NT, N], f32r, kind="ExternalInput")
    wq_d = nc.dram_tensor("wq", [128, NT, C], f32r, kind="ExternalInput")
    wk_d = nc.dram_tensor("wk", [128, NT, C], f32r, kind="ExternalInput")
    wvp_d = nc.dram_tensor("wvp", [128, NT, C], f32r, kind="ExternalInput")
    qb_d = nc.dram_tensor("qb", [128, NT], f32, kind="ExternalInput")
    kb_d = nc.dram_tensor("kb", [128, NT], f32, kind="ExternalInput")
    pb_d = nc.dram_tensor("pb", [128, NT], f32, kind="ExternalInput")
    gw_d = nc.dram_tensor("gw", [128, NT], f32, kind="ExternalInput")
    gb_d = nc.dram_tensor("gb", [128, NT], f32, kind="ExternalInput")
    ind_d = nc.dram_tensor("ind", [128, NT, GROUPS], f32, kind="ExternalInput")
    indT_d = nc.dram_tensor("indT", [GROUPS, NT, 128], f32, kind="ExternalInput")
    id_d = nc.dram_tensor("ident", [128, 128], f32, kind="ExternalInput")
    out_d = nc.dram_tensor("out", [NT, 128, CHUNK], f32, kind="ExternalOutput")

    with tile.TileContext(nc) as tc, ExitStack() as ctx:
        consts = ctx.enter_context(tc.tile_pool(name="consts", bufs=1))
        big = ctx.enter_context(tc.tile_pool(name="big", bufs=1))
        xst = ctx.enter_context(tc.tile_pool(name="xst", bufs=3))
        work = ctx.enter_context(tc.tile_pool(name="work", bufs=2))
        ptp = ctx.enter_context(tc.tile_pool(name="ptp", bufs=3))
        outp = ctx.enter_context(tc.tile_pool(name="outp", bufs=2))
        small = ctx.enter_context(tc.tile_pool(name="small", bufs=1))
        ps_s = ctx.enter_context(tc.tile_pool(name="ps_s", bufs=4, space="PSUM"))
        ps_pv = ctx.enter_context(tc.tile_pool(name="ps_pv", bufs=1, space="PSUM"))
        ps_qkv = ps_s

        # ---- constants ----
        wq_sb = consts.tile([128, NT, C], f32r)
        wk_sb = consts.tile([128, NT, C], f32r)
        wvp_sb = consts.tile([128, NT, C], f32r)
        for t_sb, t_d in ((wq_sb, wq_d), (wk_sb, wk_d), (wvp_sb, wvp_d)):
            nc.gpsimd.dma_start(t_sb[:], t_d[:, :, :])
        qb_sb = consts.tile([128, NT], f32)
        kb_sb = consts.tile([128, NT], f32)
        pb_sb = consts.tile([128, NT], f32)
        gw_sb = consts.tile([128, NT], f32)
        gb_sb = consts.tile([128, NT], f32)
        for t_sb, t_d in ((qb_sb, qb_d), (kb_sb, kb_d), (pb_sb, pb_d), (gw_sb, gw_d), (gb_sb, gb_d)):
            nc.gpsimd.dma_start(t_sb[:], t_d[:, :])
        ind_sb = consts.tile([128, NT, GROUPS], f32)
        nc.gpsimd.dma_start(ind_sb[:], ind_d[:, :, :])
        indT_sb = consts.tile([GROUPS, NT, 128], f32)
        nc.gpsimd.dma_start(indT_sb[:], indT_d[:, :, :])
        id_sb = consts.tile([128, 128], f32)
        nc.gpsimd.dma_start(id_sb[:], id_d[:, :])
        eps_t = consts.tile([GROUPS, 1], f32)
        nc.vector.memset(eps_t[:], EPS)
        # preload the ln/exp activation tables while the x stream runs
        warm = consts.tile([1, 1], f32)
        nc.scalar.activation(warm[:], eps_t[0:1, :], Act.Ln)
        nc.scalar.activation(warm[:], warm[:], Act.Exp)

        # ---- persistent big buffers ----
        k_sb = big.tile([128, NT, N], f32r)
        q_sb = big.tile([128, NT, CHUNK], f32r)
        vpT = big.tile([128, MT, C + 2], f32r)
        nc.vector.memset(vpT[:, :, C:C + 1].bitcast(f32), 1.0)
        nc.vector.memset(vpT[:, :, C + 1:C + 2].bitcast(f32), 0.0)

        # ================= Phase A: groupnorm stats =================
        SCH = 512
        NSC = N // SCH
        st_all = big.tile([128, NT, NSC, 6], f32)
        for im in reversed(range(NSC)):
            xa = xst.tile([128, NT, SCH], f32r, tag="xa", bufs=5)
            nc.sync.dma_start(xa[:], x_d[:, :, bass.ts(im, SCH)])
            for t in range(NT):
                nc.vector.bn_stats(out=st_all[:, t, im, :], in_=xa[:, t, :])
        mv = big.tile([128, NT, 2], f32)
        # per-channel (mean, E[x^2]); group-reduce via indicator matmul
        for t in range(NT):
            nc.vector.bn_aggr(out=mv[:, t, :], in_=st_all[:, t, :, :])
        sq = small.tile([128, NT, 1], f32, tag="sq")
        nc.vector.tensor_mul(sq[:], mv[:, :, 0:1], mv[:, :, 0:1])
        nc.vector.tensor_add(mv[:, :, 1:2], mv[:, :, 1:2], sq[:])
        gsps = []
        for t in range(NT):
            gsp_t = ps_qkv.tile([GROUPS, 2], f32, tag="sp", name=f"gsp{t}")
            nc.tensor.matmul(gsp_t[:], ind_sb[:, t, :], mv[:, t, :],
                             start=True, stop=True)
            gsps.append(gsp_t)
        gsum = small.tile([GROUPS, 2], f32, tag="gsum")
        nc.vector.tensor_copy(gsum[:], gsps[0][:])
        nc.vector.tensor_add(gsum[:], gsum[:], gsps[1][:])
        gm = small.tile([GROUPS, 1], f32, tag="gm")
        ge2 = small.tile([GROUPS, 1], f32, tag="ge2")
        nc.vector.tensor_scalar_mul(gm[:], gsum[:, 0:1], 1.0 / CPG)
        nc.vector.tensor_scalar_mul(ge2[:], gsum[:, 1:2], 1.0 / CPG)
        gm2 = small.tile([GROUPS, 1], f32, tag="gm2")
        nc.vector.tensor_mul(gm2[:], gm[:], gm[:])
        gvar = small.tile([GROUPS, 1], f32, tag="gvar")
        nc.vector.tensor_sub(gvar[:], ge2[:], gm2[:])
        # rstd = exp(-0.5 * ln(var + eps))  (sqrt activation is too imprecise)
        lnv = small.tile([GROUPS, 1], f32, tag="lnv")
        nc.scalar.activation(lnv[:], gvar[:], Act.Ln, bias=eps_t[:], scale=1.0)
        grs = small.tile([GROUPS, 1], f32, tag="grs")
        nc.scalar.activation(grs[:], lnv[:], Act.Exp, scale=-0.5)
        gsb = small.tile([GROUPS, 2], f32, tag="gsb")
        nc.vector.tensor_copy(gsb[:, 0:1], gm[:])
        nc.vector.tensor_copy(gsb[:, 1:2], grs[:])
        # broadcast to channels; fold into per-channel affine h = A*x + B
        ab = big.tile([128, NT, 2], f32)
        for t in range(NT):
            mrp = ps_qkv.tile([128, 2], f32, tag="sp")
            nc.tensor.matmul(mrp[:], indT_sb[:, t, :], gsb[:], start=True, stop=True)
            tmp = small.tile([128, 1], f32, tag="tmpab")
            nc.vector.tensor_mul(ab[:, t, 0:1], mrp[:, 1:2], gw_sb[:, t:t + 1])
            nc.vector.tensor_mul(tmp[:], mrp[:, 0:1], ab[:, t, 0:1])
            nc.vector.tensor_sub(ab[:, t, 1:2], gb_sb[:, t:t + 1], tmp[:])

        # ================= Phase B: h -> k, q, v'^T =================
        def load_and_normalize(im):
            xt = xst.tile([128, NT, MACRO], f32r, tag="xa", bufs=5,
                          name=f"xt_{im}")
            nc.sync.dma_start(xt[:], x_d[:, :, bass.ts(im, MACRO)])
            ht = work.tile([128, NT, MACRO], f32r, tag="ht", bufs=3,
                           name=f"ht_{im}")
            for t in range(NT):
                nc.vector.tensor_scalar(
                    out=ht[:, t, :], in0=xt[:, t, :],
                    scalar1=ab[:, t, 0:1], scalar2=ab[:, t, 1:2],
                    op0=Alu.mult, op1=Alu.add)
            return ht

        ht_next = load_and_normalize(0)
        for im in range(NMAC):
            mb = im * MACRO
            ht = ht_next
            if im + 1 < NMAC:
                ht_next = load_and_normalize(im + 1)
            # k = Wk @ h + kb   (k_sb[:, oc, :] in [o, m] layout)
            for oc in range(NT):
                kp = ps_qkv.tile([128, MACRO], f32, tag="sp")
                for t in range(NT):
                    nc.tensor.matmul(kp[:], wk_sb[:, t, bass.ts(oc, 128)],
                                     ht[:, t, :], start=(t == 0), stop=(t == NT - 1))
                nc.vector.tensor_scalar_add(
                    k_sb[:, oc, bass.ts(im, MACRO)], kp[:], kb_sb[:, oc:oc + 1])
            # q only for owned chunk (columns [0, CHUNK))
            qlo = max(mb, 0)
            qhi = min(mb + MACRO, CHUNK)
            if qlo < qhi:
                qn = qhi - qlo
                for oc in range(NT):
                    qp = ps_qkv.tile([128, MACRO], f32, tag="sp")
                    for t in range(NT):
                        nc.tensor.matmul(qp[:, :qn],
                                         wq_sb[:, t, bass.ts(oc, 128)],
                                         ht[:, t, qlo - mb:qhi - mb],
                                         start=(t == 0), stop=(t == NT - 1))
                    nc.vector.tensor_scalar_add(
                        q_sb[:, oc, qlo:qhi], qp[:, :qn], qb_sb[:, oc:oc + 1])
            # v'^T tiles: v'T[m, o] = sum_c h[c, m] * wvp[c, o]
            for mm in range(MACRO // 128):
                j = im * (MACRO // 128) + mm
                vpp = ps_qkv.tile([128, C], f32, tag="sp")
                for t in range(NT):
                    nc.tensor.matmul(vpp[:], ht[:, t, bass.ts(mm, 128)],
                                     wvp_sb[:, t, :], start=(t == 0), stop=(t == NT - 1))
                nc.scalar.copy(vpT[:, j, 0:C], vpp[:])

        # ================= Phase C: attention per n-block =================
        NH = NBLK // 128
        for blk in range(NBLOCKS):
            nb = blk * NBLK
            pvs = [ps_pv.tile([128, C + 2], f32, tag=f"pv{nh}", name=f"pv{nh}_{blk}") for nh in range(NH)]
            HB = NBLK // 2
            prev = None
            for j in range(MT):
                sp = ps_s.tile([128, NBLK], f32, tag="sp")
                for t in range(NT):
                    nc.tensor.matmul(sp[:],
                                     k_sb[:, t, bass.ts(j, 128)],
                                     q_sb[:, t, nb:nb + NBLK],
                                     start=(t == 0), stop=(t == NT - 1))
                pt = ptp.tile([128, NBLK], f32r, tag="pt", bufs=3)
                if prev is not None:
                    spp, ptq, jp = prev
                    nc.scalar.activation(ptq[:, HB:NBLK], spp[:, HB:NBLK],
                                         Act.Exp, scale=SCALE)
                    for nh in range(NH // 2, NH):
                        nc.tensor.matmul(pvs[nh][:],
                                         ptq[:, bass.ts(nh, 128)],
                                         vpT[:, jp, :],
                                         start=(jp == 0), stop=(jp == MT - 1))
                nc.scalar.activation(pt[:, 0:HB], sp[:, 0:HB],
                                     Act.Exp, scale=SCALE)
                for nh in range(NH // 2):
                    nc.tensor.matmul(pvs[nh][:],
                                     pt[:, bass.ts(nh, 128)],
                                     vpT[:, j, :],
                                     start=(j == 0), stop=(j == MT - 1))
                prev = (sp, pt, j)
            spp, ptq, jp = prev
            nc.scalar.activation(ptq[:, HB:NBLK], spp[:, HB:NBLK],
                                 Act.Exp, scale=SCALE)
            for nh in range(NH // 2, NH):
                nc.tensor.matmul(pvs[nh][:],
                                 ptq[:, bass.ts(nh, 128)],
                                 vpT[:, jp, :],
                                 start=(jp == 0), stop=(jp == MT - 1))
            # finalize: scale rows by 1/rowsum, transpose to [o, n], +bias +x
            xres = outp.tile([128, NT, NBLK], f32r, tag="xres", bufs=1)
            nc.sync.dma_start(xres[:], x_d[:, :, nb:nb + NBLK])
            for oc in range(NT):
                nc.vector.tensor_scalar_add(xres[:, oc, :].bitcast(f32),
                                            xres[:, oc, :].bitcast(f32),
                                            pb_sb[:, oc:oc + 1])
            outT = outp.tile([128, NH, C], f32, tag="outT", bufs=1)
            for nh in range(NH):
                rec = small.tile([128, 1], f32, tag="rec", bufs=2)
                nc.vector.reciprocal(rec[:], pvs[nh][:, C:C + 1])
                nc.scalar.activation(outT[:, nh, :], pvs[nh][:, 0:C],
                                     Act.Copy, scale=rec[:])
            ob = outp.tile([128, NT, NBLK], f32, tag="ob")
            for oc in range(NT):
                for nh in range(NH):
                    tp = ps_s.tile([128, 128], f32, tag="sp")
                    nc.tensor.transpose(tp[:], outT[:, nh, bass.ts(oc, 128)], id_sb[:])
                    nc.vector.tensor_add(ob[:, oc, bass.ts(nh, 128)], tp[:],
                                         xres[:, oc, bass.ts(nh, 128)].bitcast(f32))
                nc.sync.dma_start(out_d[oc, :, nb:nb + NBLK], ob[:, oc, :])

    nc.compile()
    return nc


def prep_inputs(xf_b_rot, qkv_w, qkv_b, proj_w, proj_b, norm_w, norm_b, N):
    """Per-core input map. xf_b_rot: [C, N] rotated so owned chunk is cols 0:."""
    def ct(a):  # [C, ...] -> [128, NT, ...] with channel = t*128 + p
        return np.ascontiguousarray(a.reshape(NT, 128, *a.shape[1:]).transpose(
            1, 0, *range(2, a.ndim + 1)))

    wq = qkv_w[0:C].T          # [c, o]
    wk = qkv_w[C:2 * C].T
    wvp = (proj_w.astype(np.float64) @ qkv_w[2 * C:3 * C].astype(np.float64)
           ).astype(np.float32).T
    pbm = (proj_b.astype(np.float64)
           + proj_w.astype(np.float64) @ qkv_b[2 * C:3 * C].astype(np.float64)
           ).astype(np.float32)
    ind = np.zeros((C, GROUPS), np.float32)
    ind[np.arange(C), np.arange(C) // CPG] = 1.0
    indT = np.ascontiguousarray(
        ind.reshape(NT, 128, GROUPS).transpose(2, 0, 1))  # [g, t, p]
    return {
        "x": ct(xf_b_rot.astype(np.float32)),
        "wq": ct(wq.astype(np.float32)), "wk": ct(wk.astype(np.float32)),
        "wvp": ct(wvp),
        "qb": ct(qkv_b[0:C].astype(np.float32)),
        "kb": ct(qkv_b[C:2 * C].astype(np.float32)),
        "pb": ct(pbm),
        "gw": ct(norm_w.astype(np.float32)), "gb": ct(norm_b.astype(np.float32)),
        "ind": ct(ind), "indT": indT,
        "ident": np.eye(128, dtype=np.float32),
    }


_NC_CACHE = {}


def _get_nc(N, CHUNK):
    key = (N, CHUNK)
    if key not in _NC_CACHE:
        _NC_CACHE[key] = build_nc(N=N, CHUNK=CHUNK)
    return _NC_CACHE[key]


def run_cores(inputs, N=8192, CHUNK=2048, trace=False):
    """Shard, run on 8 cores, return (out [B, C, N], BassKernelResults)."""
    from concourse.bass_utils import run_bass_kernel_spmd

    x = np.asarray(inputs["x"], np.float32)
    B = x.shape[0]
    xf = x.reshape(B, C, N)
    qshards = 8 // B
    nc = _get_nc(N, CHUNK)
    in_maps = []
    for core in range(8):
        b, ci = divmod(core, qshards)
        s = ci * CHUNK
        xrot = np.concatenate([xf[b][:, s:], xf[b][:, :s]], axis=1)
        in_maps.append(prep_inputs(
            xrot, np.asarray(inputs["qkv_w"], np.float32),
            np.asarray(inputs["qkv_b"], np.float32),
            np.asarray(inputs["proj_w"], np.float32),
            np.asarray(inputs["proj_b"], np.float32),
            np.asarray(inputs["norm_w"], np.float32),
            np.asarray(inputs["norm_b"], np.float32), N))
    res = run_bass_kernel_spmd(nc, in_maps, core_ids=list(range(8)), trace=trace)
    out = np.empty((B, C, N), np.float32)
    for core in range(8):
        b, ci = divmod(core, qshards)
        out[b][:, ci * CHUNK:(ci + 1) * CHUNK] = res.results[core]["out"].reshape(
            C, CHUNK)
    return out, res


def kernel(x, norm_w, norm_b, qkv_w, qkv_b, proj_w, proj_b):
    B, _, D, H, W = x.shape
    out, _ = run_cores(
        dict(x=x, norm_w=norm_w, norm_b=norm_b, qkv_w=qkv_w, qkv_b=qkv_b,
             proj_w=proj_w, proj_b=proj_b),
        N=D * H * W, CHUNK=D * H * W * B // 8)
    return out.reshape(B, C, D, H, W).astype(np.float32)
